# revision 16
# baseline (speedup 1.0000x reference)
"""Trainium2 Bass kernel for 3-layer GraphSAGE (mean aggregation).

Strategy (graph/data parallel over 8 NeuronCores):
  - Nodes are sharded contiguously: core c owns nodes [c*6250, (c+1)*6250).
  - Per layer k: every core computes m_k = h_k @ W_neigh_k.T for its own
    nodes (pre-multiplied messages, bf16, padded to 128 cols), AllGathers
    m_k into a full [50176, 128] bf16 DRAM buffer, then gathers per-edge
    source rows with dma_gather (256B rows), segment-sums them onto its
    owned destination nodes via one-hot matmuls on the PE (one-hot built
    on DVE with an iota/is_equal compare), scales by 1/deg, and adds the
    self term h_k @ W_self_k.T (+bias) computed from an on-chip transposed
    copy of h_k.
  - Graph structure (src/dst) is preprocessed on the host into per-core,
    per-destination-block edge tiles of 128, split into lo/hi halves of
    the global node-row space so gather indices fit in int16. Tile counts
    are made uniform across cores (SPMD: one NEFF for all 8 cores).
"""

import math

import numpy as np
import ml_dtypes

# problem constants (hardcoded per harness contract)
N_NODES = 50000
N_EDGES = 800000
D_IN, D_HID, D_OUT = 96, 96, 64

NC = 8  # cores
P = 128  # partitions
NPC = N_NODES // NC  # 6250 owned nodes per core
B = math.ceil(NPC / P)  # 49 dst blocks per core
NPCP = B * P  # 6272 padded nodes per core
HSPLIT = (NC // 2) * NPCP  # 25088: global row space lo/hi split
DPAD = 128  # padded message width (256B rows in bf16)
CH = 48  # gather chunk size in tiles of 128 edges

BF16 = ml_dtypes.bfloat16


def _prep_graph(src, dst):
    """Sort/pad edges into per-core, per-(block, half) tiles of 128.

    Returns per-core idx/dstid arrays plus the (core-uniform) tile counts
    Tbh[B, 2].
    """
    src = src.astype(np.int64)
    dst = dst.astype(np.int64)
    owner = dst // NPC
    dloc = dst % NPC
    blk = dloc // P
    lane = (dloc % P).astype(np.float32)
    gsrc = (src // NPC) * NPCP + (src % NPC)
    half = (gsrc >= HSPLIT).astype(np.int64)

    key = (owner * B + blk) * 2 + half
    order = np.argsort(key, kind="stable")
    gsrc_s = gsrc[order]
    lane_s = lane[order]

    counts = np.bincount(key, minlength=NC * B * 2).reshape(NC, B, 2)
    # uniform tile counts across cores; >=1 so every block has a matmul
    Tbh = np.maximum((-(-counts // P)).max(axis=0), 1)  # [B, 2]
    Tlo = int(Tbh[:, 0].sum())
    Thi = int(Tbh[:, 1].sum())
    Th = {0: Tlo, 1: Thi}
    tile_start = {
        h: np.concatenate([[0], np.cumsum(Tbh[:, h])]).astype(np.int64)
        for h in (0, 1)
    }

    starts = np.zeros(NC * B * 2 + 1, np.int64)
    np.cumsum(counts.reshape(-1), out=starts[1:])

    idx = {h: np.zeros((NC, Th[h] * P), np.int64) for h in (0, 1)}
    did = {h: np.full((NC, Th[h] * P), -1.0, np.float32) for h in (0, 1)}
    for c in range(NC):
        for b in range(B):
            for h in (0, 1):
                k = (c * B + b) * 2 + h
                s0, s1 = starts[k], starts[k + 1]
                n = s1 - s0
                off = tile_start[h][b] * P
                g = gsrc_s[s0:s1] - (HSPLIT if h else 0)
                idx[h][c, off : off + n] = g
                did[h][c, off : off + n] = lane_s[s0:s1]

    assert idx[0].max(initial=0) < 2**15 and idx[1].max(initial=0) < 2**15
    return idx, did, Tbh, tile_start, Tlo, Thi


def _wrap_idx(a):
    """[T*P] int -> dma_gather SBUF layout [128, T*8] int16 (idx i at
    [i%16, i//16], replicated to all 8 Q7-core partition groups)."""
    w = a.reshape(-1, 16).T.astype(np.int16)  # [16, T*8]
    return np.tile(w, (8, 1))


def _build_inputs_per_core(inputs):
    """Host preprocessing: shard + reorganize the problem inputs."""
    x = np.asarray(inputs["x"], np.float32)
    src = np.asarray(inputs["src"])
    dst = np.asarray(inputs["dst"])

    idx, did, Tbh, tile_start, Tlo, Thi = _prep_graph(src, dst)

    deg = np.zeros(N_NODES, np.float32)
    np.add.at(deg, dst, 1.0)
    recip = 1.0 / np.maximum(deg, 1.0)

    # weights: pre-transposed, bf16; self gets bias row appended
    wn = []
    ws = []
    for i, dout in enumerate((D_HID, D_HID, D_OUT)):
        wn.append(np.ascontiguousarray(inputs[f"w_neigh{i}"].T).astype(BF16))
        wst = np.concatenate(
            [inputs[f"w_self{i}"].T, inputs[f"b{i}"][None, :]], axis=0
        ).astype(BF16)
        ws.append(np.ascontiguousarray(wst))

    per_core = []
    for c in range(NC):
        xc = x[c * NPC : (c + 1) * NPC]
        xT = np.zeros((97, NPCP), BF16)
        xT[:96, :NPC] = xc.T.astype(BF16)
        xT[96, :] = 1.0  # ones row for the bias term
        rc = np.zeros((P, B), np.float32)
        rflat = np.zeros(NPCP, np.float32)
        rflat[:NPC] = recip[c * NPC : (c + 1) * NPC]
        rc[:, :] = rflat.reshape(B, P).T
        m = {
            "xT": xT,
            "idx_lo": _wrap_idx(idx[0][c]),
            "idx_hi": _wrap_idx(idx[1][c]),
            "dstid_lo": np.ascontiguousarray(
                did[0][c].reshape(Tlo, P).T.astype(BF16)
            ),
            "dstid_hi": np.ascontiguousarray(
                did[1][c].reshape(Thi, P).T.astype(BF16)
            ),
            "recip": rc,
        }
        for i in range(3):
            m[f"wn{i}"] = wn[i]
            m[f"ws{i}"] = ws[i]
        per_core.append(m)
    return per_core, Tbh, tile_start, Tlo, Thi


def _chunks(T):
    """Split T tiles into chunks of <=CH tiles: list of (start, count)."""
    out = []
    t = 0
    while t < T:
        ct = min(CH, T - t)
        out.append((t, ct))
        t += ct
    return out


def _build_bass(Tbh, tile_start, Tlo, Thi, reps=1):
    import concourse.bass as bass
    import concourse.bacc as bacc
    import concourse.mybir as mybir
    import concourse.tile as tile

    dt = mybir.dt
    Alu = mybir.AluOpType
    Act = mybir.ActivationFunctionType

    nc = bacc.Bacc(
        "TRN2",
        target_bir_lowering=False,
        debug=False,
        num_devices=NC,
        num_swdge_queues=2,
    )

    # ---- I/O ----
    xT_d = nc.dram_tensor("xT", [97, NPCP], dt.bfloat16, kind="ExternalInput")
    idx_d = {
        0: nc.dram_tensor("idx_lo", [P, Tlo * 8], dt.int16, kind="ExternalInput"),
        1: nc.dram_tensor("idx_hi", [P, Thi * 8], dt.int16, kind="ExternalInput"),
    }
    did_d = {
        0: nc.dram_tensor("dstid_lo", [P, Tlo], dt.bfloat16, kind="ExternalInput"),
        1: nc.dram_tensor("dstid_hi", [P, Thi], dt.bfloat16, kind="ExternalInput"),
    }
    recip_d = nc.dram_tensor("recip", [P, B], dt.float32, kind="ExternalInput")
    wn_d = []
    ws_d = []
    for i, dout in enumerate((D_HID, D_HID, D_OUT)):
        wn_d.append(
            nc.dram_tensor(f"wn{i}", [96, dout], dt.bfloat16, kind="ExternalInput")
        )
        ws_d.append(
            nc.dram_tensor(f"ws{i}", [97, dout], dt.bfloat16, kind="ExternalInput")
        )
    out_d = nc.dram_tensor("out", [NPCP, D_OUT], dt.float32, kind="ExternalOutput")

    ident_np = np.eye(P, dtype=BF16)
    ident_d = nc.inline_tensor(ident_np, "ident")
    iota_np = np.tile(np.arange(P, dtype=BF16)[None, :], (P, 1))
    iota_d = nc.inline_tensor(iota_np, "iota")

    # internal DRAM
    m_bounce = nc.dram_tensor("m_bounce", [NPCP, DPAD], dt.bfloat16)
    m_full = nc.dram_tensor(
        "m_full", [NC * NPCP, DPAD], dt.bfloat16, addr_space="Shared"
    )

    # ---- persistent SBUF ----
    hT = nc.alloc_sbuf_tensor("hT", [128, NPCP], dt.bfloat16)  # rows 0:97 used
    h_own = nc.alloc_sbuf_tensor("h_own", [P, B * 96], dt.bfloat16)
    m_big = nc.alloc_sbuf_tensor("m_big", [P, B * DPAD], dt.bfloat16)
    out_big = nc.alloc_sbuf_tensor("out_big", [P, B * D_OUT], dt.float32)
    idx_sb = {
        0: nc.alloc_sbuf_tensor("idx_lo_sb", [P, Tlo * 8], dt.int16),
        1: nc.alloc_sbuf_tensor("idx_hi_sb", [P, Thi * 8], dt.int16),
    }
    did_sb = {
        0: nc.alloc_sbuf_tensor("did_lo_sb", [P, Tlo], dt.bfloat16),
        1: nc.alloc_sbuf_tensor("did_hi_sb", [P, Thi], dt.bfloat16),
    }
    recip_sb = nc.alloc_sbuf_tensor("recip_sb", [P, B], dt.float32)
    ident_sb = nc.alloc_sbuf_tensor("ident_sb", [P, P], dt.bfloat16)
    iota_sb = nc.alloc_sbuf_tensor("iota_sb", [P, P], dt.bfloat16)
    wn_sb = []
    ws_sb = []
    for i, dout in enumerate((D_HID, D_HID, D_OUT)):
        wn_sb.append(nc.alloc_sbuf_tensor(f"wn{i}_sb", [96, dout], dt.bfloat16))
        ws_sb.append(nc.alloc_sbuf_tensor(f"ws{i}_sb", [97, dout], dt.bfloat16))

    douts = (D_HID, D_HID, D_OUT)
    ch_plan = {0: _chunks(Tlo), 1: _chunks(Thi)}

    with tile.TileContext(nc) as tc:
        with (
            tc.tile_pool(name="sb", bufs=2) as sb,
            tc.tile_pool(name="ps", bufs=2, space="PSUM") as ps,
        ):
            # ---- load constants / inputs to SBUF ----
            nc.vector.memset(m_big.ap(), 0)
            nc.sync.dma_start(out=hT[:97, :], in_=xT_d[:, :])
            for h in (0, 1):
                nc.sync.dma_start(out=idx_sb[h].ap(), in_=idx_d[h][:, :])
                nc.sync.dma_start(out=did_sb[h].ap(), in_=did_d[h][:, :])
            nc.sync.dma_start(out=recip_sb.ap(), in_=recip_d[:, :])
            nc.sync.dma_start(out=ident_sb.ap(), in_=ident_d[:, :])
            nc.sync.dma_start(out=iota_sb.ap(), in_=iota_d[:, :])
            for i in range(3):
                nc.sync.dma_start(out=wn_sb[i].ap(), in_=wn_d[i][:, :])
                nc.sync.dma_start(out=ws_sb[i].ap(), in_=ws_d[i][:, :])

            for rep in range(reps):
              for k in range(3):
                dout = douts[k]
                # ---- phase A: hT (layers 1,2) and m_own ----
                if k > 0 or rep > 0:
                    for b in range(B):
                        tp = ps.tile([96, P], dt.bfloat16, tag="tp")
                        nc.tensor.transpose(
                            tp[:],
                            h_own.ap()[:, b * 96 : b * 96 + 96],
                            ident_sb.ap(),
                        )
                        nc.scalar.copy(
                            out=hT[:96, b * P : (b + 1) * P], in_=tp[:]
                        )
                for b in range(B):
                    mp = ps.tile([P, dout], dt.float32, tag="pm")
                    nc.tensor.matmul(
                        mp[:],
                        hT[:96, b * P : (b + 1) * P],
                        wn_sb[k].ap(),
                        start=True,
                        stop=True,
                    )
                    nc.scalar.copy(
                        out=m_big.ap()[:, b * DPAD : b * DPAD + dout], in_=mp[:]
                    )
                # m_own -> DRAM bounce -> AllGather
                import concourse.bass as _b
                nc.sync.dma_start(
                    out=bass.AP(
                        tensor=m_bounce.ap().tensor,
                        offset=0,
                        ap=[[DPAD, P], [DPAD * P, B], [1, DPAD]],
                    ),
                    in_=m_big.ap(),
                )
                nc.gpsimd.collective_compute(
                    "AllGather",
                    Alu.bypass,
                    replica_groups=[list(range(NC))],
                    ins=[m_bounce.ap().opt()],
                    outs=[m_full.ap().opt()],
                )

                # ---- phase B+C: gather chunks lazily, aggregate per block ----
                mview = {
                    0: m_full[0:HSPLIT, :],
                    1: m_full[HSPLIT : 2 * HSPLIT, :],
                }
                msgs_t = {0: {}, 1: {}}
                oh_t = {0: {}, 1: {}}
                emitted = {0: 0, 1: 0}

                def emit_chunk(h):
                    ci = emitted[h]
                    t0, ct = ch_plan[h][ci]
                    ms = sb.tile([P, ct, DPAD], dt.bfloat16, tag=f"msgs{h}")
                    nc.gpsimd.dma_gather(
                        ms[:],
                        mview[h],
                        idx_sb[h][:, t0 * 8 : (t0 + ct) * 8],
                        ct * P,
                        ct * P,
                        DPAD,
                        queue_num=h,
                        single_packet=False,
                    )
                    msgs_t[h][ci] = ms
                    oh = sb.tile([P, ct * P], dt.bfloat16, tag=f"oh{h}")
                    oh3 = oh[:].rearrange("p (t j) -> p t j", j=P)
                    nc.vector.tensor_tensor(
                        out=oh3,
                        in0=did_sb[h][:, t0 : t0 + ct].to_broadcast([P, ct, P]),
                        in1=bass.AP(
                            tensor=iota_sb.ap().tensor,
                            offset=0,
                            ap=[[P, P], [0, ct], [1, P]],
                        ),
                        op=Alu.is_equal,
                    )
                    oh_t[h][ci] = oh
                    emitted[h] = ci + 1

                for b in range(B):
                    # make sure every chunk this block touches is emitted
                    for h in (0, 1):
                        last_tile = tile_start[h][b + 1] - 1
                        while emitted[h] <= last_tile // CH:
                            emit_chunk(h)
                    pagg = ps.tile([P, dout], dt.float32, tag="pagg")
                    tiles = []
                    for h in (0, 1):
                        for o in range(tile_start[h][b], tile_start[h][b + 1]):
                            tiles.append((h, o // CH, o % CH))
                    for j, (h, ci, off) in enumerate(tiles):
                        nc.tensor.matmul(
                            pagg[:],
                            oh_t[h][ci][:, off * P : (off + 1) * P],
                            msgs_t[h][ci][:, off, :dout],
                            start=(j == 0),
                            stop=(j == len(tiles) - 1),
                        )
                    pself = ps.tile([P, dout], dt.float32, tag="pself")
                    nc.tensor.matmul(
                        pself[:],
                        hT[:97, b * P : (b + 1) * P],
                        ws_sb[k].ap(),
                        start=True,
                        stop=True,
                    )
                    selfsb = sb.tile([P, dout], dt.float32, tag="selfsb")
                    nc.scalar.copy(out=selfsb[:], in_=pself[:])
                    if k < 2:
                        t1 = sb.tile([P, dout], dt.bfloat16, tag="t1")
                        nc.vector.scalar_tensor_tensor(
                            out=t1[:],
                            in0=pagg[:],
                            scalar=recip_sb[:, b : b + 1],
                            in1=selfsb[:],
                            op0=Alu.mult,
                            op1=Alu.add,
                        )
                        nc.scalar.activation(
                            out=h_own.ap()[:, b * 96 : b * 96 + 96],
                            in_=t1[:],
                            func=Act.Relu,
                        )
                    else:
                        nc.vector.scalar_tensor_tensor(
                            out=out_big.ap()[:, b * D_OUT : (b + 1) * D_OUT],
                            in0=pagg[:],
                            scalar=recip_sb[:, b : b + 1],
                            in1=selfsb[:],
                            op0=Alu.mult,
                            op1=Alu.add,
                        )

            nc.sync.dma_start(
                out=bass.AP(
                    tensor=out_d.ap().tensor,
                    offset=0,
                    ap=[[D_OUT, P], [D_OUT * P, B], [1, D_OUT]],
                ),
                in_=out_big.ap(),
            )

    nc.finalize()
    return nc


def _exec_timed(nc, in_maps, n_timing=12):
    """Execute a prebuilt SPMD Bass module on 8 cores with device-resident
    inputs, returning the min wall-clock seconds over n_timing calls."""
    import time

    import jax
    import concourse.mybir as mybir
    from concourse import bass2jax
    from jax.experimental.shard_map import shard_map
    from jax.sharding import Mesh, NamedSharding, PartitionSpec

    bass2jax.install_neuronx_cc_hook()

    partition_name = (
        nc.partition_id_tensor.name if nc.partition_id_tensor else None
    )
    in_names, out_names, out_avals, zero_outs = [], [], [], []
    for alloc in nc.m.functions[0].allocations:
        if not isinstance(alloc, mybir.MemoryLocationSet):
            continue
        name = alloc.memorylocations[0].name
        if alloc.kind == "ExternalInput":
            if name != partition_name:
                in_names.append(name)
        elif alloc.kind == "ExternalOutput":
            dtype = mybir.dt.np(alloc.dtype)
            out_avals.append(
                jax.core.ShapedArray(tuple(alloc.tensor_shape), dtype)
            )
            out_names.append(name)
            zero_outs.append(np.zeros(tuple(alloc.tensor_shape), dtype))
    n_params = len(in_names)
    all_in_names = in_names + out_names
    if partition_name is not None:
        all_in_names = all_in_names + [partition_name]

    def _body(*args):
        operands = list(args)
        if partition_name is not None:
            operands.append(bass2jax.partition_id_tensor())
        outs = bass2jax._bass_exec_p.bind(
            *operands,
            out_avals=tuple(out_avals),
            in_names=tuple(all_in_names),
            out_names=tuple(out_names),
            lowering_input_output_aliases=(),
            sim_require_finite=True,
            sim_require_nnan=True,
            nc=nc,
        )
        return tuple(outs)

    devices = jax.devices()[:NC]
    mesh = Mesh(np.asarray(devices), ("core",))
    nin = n_params + len(zero_outs)
    f = jax.jit(
        shard_map(
            _body,
            mesh=mesh,
            in_specs=(PartitionSpec("core"),) * nin,
            out_specs=(PartitionSpec("core"),) * len(out_names),
            check_rep=False,
        ),
        keep_unused=True,
    )
    concat_in = [
        np.concatenate([np.asarray(m[name]) for m in in_maps], axis=0)
        for name in in_names
    ] + [np.zeros((NC * z.shape[0], *z.shape[1:]), z.dtype) for z in zero_outs]
    sh = NamedSharding(mesh, PartitionSpec("core"))
    dev_in = [jax.device_put(a, sh) for a in concat_in]
    r = f(*dev_in)
    jax.block_until_ready(r)
    times = []
    for _ in range(n_timing):
        t0 = time.perf_counter()
        r = f(*dev_in)
        jax.block_until_ready(r)
        times.append(time.perf_counter() - t0)
    return min(times)


def _timed_run(inputs, rep_pair=(1, 3), n_timing=12):
    """Estimate per-iteration HW time by differencing wall times of NEFFs
    that run the whole 3-layer pipeline `r` times."""
    per_core, Tbh, tile_start, Tlo, Thi = _build_inputs_per_core(inputs)
    walls = {}
    for r in rep_pair:
        ncr = _build_bass(Tbh, tile_start, Tlo, Thi, reps=r)
        walls[r] = _exec_timed(ncr, per_core, n_timing)
    r0, r1 = rep_pair
    ns = (walls[r1] - walls[r0]) / (r1 - r0) * 1e9
    print(f"walls: {walls}", flush=True)
    return int(ns)


def _run(inputs, trace=False):
    from concourse import bass_utils

    per_core, Tbh, tile_start, Tlo, Thi = _build_inputs_per_core(inputs)
    nc = _build_bass(Tbh, tile_start, Tlo, Thi)
    res = bass_utils.run_bass_kernel_spmd(
        nc,
        per_core,
        core_ids=list(range(NC)),
        trace=trace,
    )
    outs = [r["out"][:NPC] for r in res.results]
    full = np.concatenate(outs, axis=0).astype(np.float32)
    return full, res


def kernel(**inputs):
    out, _ = _run(inputs, trace=False)
    return out


# revision 17
# speedup vs baseline: 2.5679x; 2.5679x over previous
"""Trainium2 Bass kernel for 3-layer GraphSAGE (mean aggregation).

Strategy (graph/data parallel over 8 NeuronCores):
  - Nodes are sharded contiguously: core c owns nodes [c*6250, (c+1)*6250).
  - Per layer k: every core computes m_k = h_k @ W_neigh_k.T for its own
    nodes (pre-multiplied messages, bf16, padded to 128 cols), AllGathers
    m_k into a full [50176, 128] bf16 DRAM buffer, then gathers per-edge
    source rows with dma_gather (256B rows), segment-sums them onto its
    owned destination nodes via one-hot matmuls on the PE (one-hot built
    on DVE with an iota/is_equal compare), scales by 1/deg, and adds the
    self term h_k @ W_self_k.T (+bias) computed from an on-chip transposed
    copy of h_k.
  - Graph structure (src/dst) is preprocessed on the host into per-core,
    per-destination-block edge tiles of 128, split into lo/hi halves of
    the global node-row space so gather indices fit in int16. Tile counts
    are made uniform across cores (SPMD: one NEFF for all 8 cores).
"""

import math

import numpy as np
import ml_dtypes

# problem constants (hardcoded per harness contract)
N_NODES = 50000
N_EDGES = 800000
D_IN, D_HID, D_OUT = 96, 96, 64

NC = 8  # cores
P = 128  # partitions
NPC = N_NODES // NC  # 6250 owned nodes per core
B = math.ceil(NPC / P)  # 49 dst blocks per core
NPCP = B * P  # 6272 padded nodes per core
HSPLIT = (NC // 2) * NPCP  # 25088: global row space lo/hi split
DPAD = 128  # padded message width (256B rows in bf16)
CH = 48  # gather chunk size in tiles of 128 edges

BF16 = ml_dtypes.bfloat16


def _prep_graph(src, dst):
    """Sort/pad edges into per-core, per-(block, half) tiles of 128.

    Returns per-core idx/dstid arrays plus the (core-uniform) tile counts
    Tbh[B, 2].
    """
    src = src.astype(np.int64)
    dst = dst.astype(np.int64)
    owner = dst // NPC
    dloc = dst % NPC
    blk = dloc // P
    lane = (dloc % P).astype(np.float32)
    gsrc = (src // NPC) * NPCP + (src % NPC)
    half = (gsrc >= HSPLIT).astype(np.int64)

    key = (owner * B + blk) * 2 + half
    order = np.argsort(key, kind="stable")
    gsrc_s = gsrc[order]
    lane_s = lane[order]

    counts = np.bincount(key, minlength=NC * B * 2).reshape(NC, B, 2)
    # uniform tile counts across cores; >=1 so every block has a matmul
    Tbh = np.maximum((-(-counts // P)).max(axis=0), 1)  # [B, 2]
    Tlo = int(Tbh[:, 0].sum())
    Thi = int(Tbh[:, 1].sum())
    Th = {0: Tlo, 1: Thi}
    tile_start = {
        h: np.concatenate([[0], np.cumsum(Tbh[:, h])]).astype(np.int64)
        for h in (0, 1)
    }

    starts = np.zeros(NC * B * 2 + 1, np.int64)
    np.cumsum(counts.reshape(-1), out=starts[1:])

    idx = {h: np.zeros((NC, Th[h] * P), np.int64) for h in (0, 1)}
    did = {h: np.full((NC, Th[h] * P), -1.0, np.float32) for h in (0, 1)}
    for c in range(NC):
        for b in range(B):
            for h in (0, 1):
                k = (c * B + b) * 2 + h
                s0, s1 = starts[k], starts[k + 1]
                n = s1 - s0
                off = tile_start[h][b] * P
                g = gsrc_s[s0:s1] - (HSPLIT if h else 0)
                idx[h][c, off : off + n] = g
                did[h][c, off : off + n] = lane_s[s0:s1]

    assert idx[0].max(initial=0) < 2**15 and idx[1].max(initial=0) < 2**15
    return idx, did, Tbh, tile_start, Tlo, Thi


def _wrap_idx(a):
    """[T*P] int -> dma_gather SBUF layout [128, T*8] int16 (idx i at
    [i%16, i//16], replicated to all 8 Q7-core partition groups)."""
    w = a.reshape(-1, 16).T.astype(np.int16)  # [16, T*8]
    return np.tile(w, (8, 1))


def _build_inputs_per_core(inputs):
    """Host preprocessing: shard + reorganize the problem inputs."""
    x = np.asarray(inputs["x"], np.float32)
    src = np.asarray(inputs["src"])
    dst = np.asarray(inputs["dst"])

    idx, did, Tbh, tile_start, Tlo, Thi = _prep_graph(src, dst)

    deg = np.zeros(N_NODES, np.float32)
    np.add.at(deg, dst, 1.0)
    recip = 1.0 / np.maximum(deg, 1.0)

    # weights: pre-transposed, bf16; self gets bias row appended
    wn = []
    ws = []
    for i, dout in enumerate((D_HID, D_HID, D_OUT)):
        wn.append(np.ascontiguousarray(inputs[f"w_neigh{i}"].T).astype(BF16))
        wst = np.concatenate(
            [inputs[f"w_self{i}"].T, inputs[f"b{i}"][None, :]], axis=0
        ).astype(BF16)
        ws.append(np.ascontiguousarray(wst))

    per_core = []
    for c in range(NC):
        xc = x[c * NPC : (c + 1) * NPC]
        xT = np.zeros((97, NPCP), BF16)
        xT[:96, :NPC] = xc.T.astype(BF16)
        xT[96, :] = 1.0  # ones row for the bias term
        rc = np.zeros((P, B), np.float32)
        rflat = np.zeros(NPCP, np.float32)
        rflat[:NPC] = recip[c * NPC : (c + 1) * NPC]
        rc[:, :] = rflat.reshape(B, P).T
        m = {
            "xT": xT,
            "idx_lo": _wrap_idx(idx[0][c]),
            "idx_hi": _wrap_idx(idx[1][c]),
            "dstid_lo": np.ascontiguousarray(
                did[0][c].reshape(Tlo, P).T.astype(BF16)
            ),
            "dstid_hi": np.ascontiguousarray(
                did[1][c].reshape(Thi, P).T.astype(BF16)
            ),
            "recip": rc,
        }
        for i in range(3):
            m[f"wn{i}"] = wn[i]
            m[f"ws{i}"] = ws[i]
        per_core.append(m)
    return per_core, Tbh, tile_start, Tlo, Thi


def _chunks(T):
    """Split T tiles into chunks of <=CH tiles: list of (start, count)."""
    out = []
    t = 0
    while t < T:
        ct = min(CH, T - t)
        out.append((t, ct))
        t += ct
    return out


def _build_bass(Tbh, tile_start, Tlo, Thi, reps=1, ablate=()):
    import concourse.bass as bass
    import concourse.bacc as bacc
    import concourse.mybir as mybir
    import concourse.tile as tile

    dt = mybir.dt
    Alu = mybir.AluOpType
    Act = mybir.ActivationFunctionType

    nc = bacc.Bacc(
        "TRN2",
        target_bir_lowering=False,
        debug=False,
        num_devices=NC,
        num_swdge_queues=2,
    )

    # ---- I/O ----
    xT_d = nc.dram_tensor("xT", [97, NPCP], dt.bfloat16, kind="ExternalInput")
    idx_d = {
        0: nc.dram_tensor("idx_lo", [P, Tlo * 8], dt.int16, kind="ExternalInput"),
        1: nc.dram_tensor("idx_hi", [P, Thi * 8], dt.int16, kind="ExternalInput"),
    }
    did_d = {
        0: nc.dram_tensor("dstid_lo", [P, Tlo], dt.bfloat16, kind="ExternalInput"),
        1: nc.dram_tensor("dstid_hi", [P, Thi], dt.bfloat16, kind="ExternalInput"),
    }
    recip_d = nc.dram_tensor("recip", [P, B], dt.float32, kind="ExternalInput")
    wn_d = []
    ws_d = []
    for i, dout in enumerate((D_HID, D_HID, D_OUT)):
        wn_d.append(
            nc.dram_tensor(f"wn{i}", [96, dout], dt.bfloat16, kind="ExternalInput")
        )
        ws_d.append(
            nc.dram_tensor(f"ws{i}", [97, dout], dt.bfloat16, kind="ExternalInput")
        )
    out_d = nc.dram_tensor("out", [NPCP, D_OUT], dt.float32, kind="ExternalOutput")

    ident_np = np.eye(P, dtype=BF16)
    ident_d = nc.inline_tensor(ident_np, "ident")
    iota_np = np.tile(np.arange(P, dtype=BF16)[None, :], (P, 1))
    iota_d = nc.inline_tensor(iota_np, "iota")

    # internal DRAM
    m_bounce = nc.dram_tensor("m_bounce", [NPCP, DPAD], dt.bfloat16)
    m_full = nc.dram_tensor(
        "m_full", [NC * NPCP, DPAD], dt.bfloat16, addr_space="Shared"
    )

    # ---- persistent SBUF ----
    hT = nc.alloc_sbuf_tensor("hT", [128, NPCP], dt.bfloat16)  # rows 0:97 used
    h_own = nc.alloc_sbuf_tensor("h_own", [P, B * 96], dt.bfloat16)
    m_big = nc.alloc_sbuf_tensor("m_big", [P, B * DPAD], dt.bfloat16)
    out_big = nc.alloc_sbuf_tensor("out_big", [P, B * D_OUT], dt.float32)
    idx_sb = {
        0: nc.alloc_sbuf_tensor("idx_lo_sb", [P, Tlo * 8], dt.int16),
        1: nc.alloc_sbuf_tensor("idx_hi_sb", [P, Thi * 8], dt.int16),
    }
    did_sb = {
        0: nc.alloc_sbuf_tensor("did_lo_sb", [P, Tlo], dt.bfloat16),
        1: nc.alloc_sbuf_tensor("did_hi_sb", [P, Thi], dt.bfloat16),
    }
    recip_sb = nc.alloc_sbuf_tensor("recip_sb", [P, B], dt.float32)
    ident_sb = nc.alloc_sbuf_tensor("ident_sb", [P, P], dt.bfloat16)
    iota_sb = nc.alloc_sbuf_tensor("iota_sb", [P, P], dt.bfloat16)
    wn_sb = []
    ws_sb = []
    for i, dout in enumerate((D_HID, D_HID, D_OUT)):
        wn_sb.append(nc.alloc_sbuf_tensor(f"wn{i}_sb", [96, dout], dt.bfloat16))
        ws_sb.append(nc.alloc_sbuf_tensor(f"ws{i}_sb", [97, dout], dt.bfloat16))

    douts = (D_HID, D_HID, D_OUT)
    ch_plan = {0: _chunks(Tlo), 1: _chunks(Thi)}

    with tile.TileContext(nc) as tc:
        with (
            tc.tile_pool(name="sb", bufs=2) as sb,
            tc.tile_pool(name="ps", bufs=2, space="PSUM") as ps,
        ):
            # ---- load constants / inputs to SBUF ----
            nc.vector.memset(m_big.ap(), 0)
            nc.sync.dma_start(out=hT[:97, :], in_=xT_d[:, :])
            for h in (0, 1):
                nc.sync.dma_start(out=idx_sb[h].ap(), in_=idx_d[h][:, :])
                nc.sync.dma_start(out=did_sb[h].ap(), in_=did_d[h][:, :])
            nc.sync.dma_start(out=recip_sb.ap(), in_=recip_d[:, :])
            nc.sync.dma_start(out=ident_sb.ap(), in_=ident_d[:, :])
            nc.sync.dma_start(out=iota_sb.ap(), in_=iota_d[:, :])
            for i in range(3):
                nc.sync.dma_start(out=wn_sb[i].ap(), in_=wn_d[i][:, :])
                nc.sync.dma_start(out=ws_sb[i].ap(), in_=ws_d[i][:, :])

            for rep in range(reps):
              for k in range(3):
                dout = douts[k]
                # ---- phase A: hT (layers 1,2) and m_own ----
                if k > 0 or rep > 0:
                    for b in range(B):
                        tp = ps.tile([96, P], dt.bfloat16, tag="tp")
                        nc.tensor.transpose(
                            tp[:],
                            h_own.ap()[:, b * 96 : b * 96 + 96],
                            ident_sb.ap(),
                        )
                        nc.scalar.copy(
                            out=hT[:96, b * P : (b + 1) * P], in_=tp[:]
                        )
                for b in range(B):
                    mp = ps.tile([P, dout], dt.float32, tag="pm")
                    nc.tensor.matmul(
                        mp[:],
                        hT[:96, b * P : (b + 1) * P],
                        wn_sb[k].ap(),
                        start=True,
                        stop=True,
                    )
                    nc.scalar.copy(
                        out=m_big.ap()[:, b * DPAD : b * DPAD + dout], in_=mp[:]
                    )
                # m_own -> DRAM bounce -> AllGather
                import concourse.bass as _b
                nc.sync.dma_start(
                    out=bass.AP(
                        tensor=m_bounce.ap().tensor,
                        offset=0,
                        ap=[[DPAD, P], [DPAD * P, B], [1, DPAD]],
                    ),
                    in_=m_big.ap(),
                )
                if "ag" in ablate:
                    pass
                else:
                  nc.gpsimd.collective_compute(
                    "AllGather",
                    Alu.bypass,
                    replica_groups=[list(range(NC))],
                    ins=[m_bounce.ap().opt()],
                    outs=[m_full.ap().opt()],
                )

                # ---- phase B+C: gather chunks lazily, aggregate per block ----
                mview = {
                    0: m_full[0:HSPLIT, :],
                    1: m_full[HSPLIT : 2 * HSPLIT, :],
                }
                msgs_t = {0: {}, 1: {}}
                oh_t = {0: {}, 1: {}}
                emitted = {0: 0, 1: 0}

                def emit_chunk(h):
                    ci = emitted[h]
                    t0, ct = ch_plan[h][ci]
                    ms = sb.tile([P, ct, DPAD], dt.bfloat16, tag=f"msgs{h}")
                    if "gather" in ablate:
                        nc.vector.memset(ms[:1, :1, :8], 0)
                    else:
                      nc.gpsimd.dma_gather(
                        ms[:],
                        mview[h],
                        idx_sb[h][:, t0 * 8 : (t0 + ct) * 8],
                        ct * P,
                        ct * P,
                        DPAD,
                        queue_num=h,
                        single_packet=False,
                      )
                    msgs_t[h][ci] = ms
                    oh = sb.tile([P, ct * P], dt.bfloat16, tag=f"oh{h}")
                    oh3 = oh[:].rearrange("p (t j) -> p t j", j=P)
                    nc.vector.tensor_tensor(
                        out=oh3,
                        in0=did_sb[h][:, t0 : t0 + ct].to_broadcast([P, ct, P]),
                        in1=bass.AP(
                            tensor=iota_sb.ap().tensor,
                            offset=0,
                            ap=[[P, P], [0, ct], [1, P]],
                        ),
                        op=Alu.is_equal,
                    )
                    oh_t[h][ci] = oh
                    emitted[h] = ci + 1

                for b in range(B):
                    # make sure every chunk this block touches is emitted
                    for h in (0, 1):
                        last_tile = tile_start[h][b + 1] - 1
                        while emitted[h] <= last_tile // CH:
                            emit_chunk(h)
                    pagg = ps.tile([P, dout], dt.float32, tag="pagg")
                    tiles = []
                    for h in (0, 1):
                        for o in range(tile_start[h][b], tile_start[h][b + 1]):
                            tiles.append((h, o // CH, o % CH))
                    for j, (h, ci, off) in enumerate(tiles):
                        nc.tensor.matmul(
                            pagg[:],
                            oh_t[h][ci][:, off * P : (off + 1) * P],
                            msgs_t[h][ci][:, off, :dout],
                            start=(j == 0),
                            stop=(j == len(tiles) - 1),
                        )
                    pself = ps.tile([P, dout], dt.float32, tag="pself")
                    nc.tensor.matmul(
                        pself[:],
                        hT[:97, b * P : (b + 1) * P],
                        ws_sb[k].ap(),
                        start=True,
                        stop=True,
                    )
                    selfsb = sb.tile([P, dout], dt.float32, tag="selfsb")
                    nc.scalar.copy(out=selfsb[:], in_=pself[:])
                    if k < 2:
                        t1 = sb.tile([P, dout], dt.bfloat16, tag="t1")
                        nc.vector.scalar_tensor_tensor(
                            out=t1[:],
                            in0=pagg[:],
                            scalar=recip_sb[:, b : b + 1],
                            in1=selfsb[:],
                            op0=Alu.mult,
                            op1=Alu.add,
                        )
                        nc.scalar.activation(
                            out=h_own.ap()[:, b * 96 : b * 96 + 96],
                            in_=t1[:],
                            func=Act.Relu,
                        )
                    else:
                        nc.vector.scalar_tensor_tensor(
                            out=out_big.ap()[:, b * D_OUT : (b + 1) * D_OUT],
                            in0=pagg[:],
                            scalar=recip_sb[:, b : b + 1],
                            in1=selfsb[:],
                            op0=Alu.mult,
                            op1=Alu.add,
                        )

            nc.sync.dma_start(
                out=bass.AP(
                    tensor=out_d.ap().tensor,
                    offset=0,
                    ap=[[D_OUT, P], [D_OUT * P, B], [1, D_OUT]],
                ),
                in_=out_big.ap(),
            )

    nc.finalize()
    return nc


def _exec_timed(nc, in_maps, n_timing=12):
    """Execute a prebuilt SPMD Bass module on 8 cores with device-resident
    inputs, returning the min wall-clock seconds over n_timing calls."""
    import time

    import jax
    import concourse.mybir as mybir
    from concourse import bass2jax
    from jax.experimental.shard_map import shard_map
    from jax.sharding import Mesh, NamedSharding, PartitionSpec

    bass2jax.install_neuronx_cc_hook()

    partition_name = (
        nc.partition_id_tensor.name if nc.partition_id_tensor else None
    )
    in_names, out_names, out_avals, zero_outs = [], [], [], []
    for alloc in nc.m.functions[0].allocations:
        if not isinstance(alloc, mybir.MemoryLocationSet):
            continue
        name = alloc.memorylocations[0].name
        if alloc.kind == "ExternalInput":
            if name != partition_name:
                in_names.append(name)
        elif alloc.kind == "ExternalOutput":
            dtype = mybir.dt.np(alloc.dtype)
            out_avals.append(
                jax.core.ShapedArray(tuple(alloc.tensor_shape), dtype)
            )
            out_names.append(name)
            zero_outs.append(np.zeros(tuple(alloc.tensor_shape), dtype))
    n_params = len(in_names)
    all_in_names = in_names + out_names
    if partition_name is not None:
        all_in_names = all_in_names + [partition_name]

    def _body(*args):
        operands = list(args)
        if partition_name is not None:
            operands.append(bass2jax.partition_id_tensor())
        outs = bass2jax._bass_exec_p.bind(
            *operands,
            out_avals=tuple(out_avals),
            in_names=tuple(all_in_names),
            out_names=tuple(out_names),
            lowering_input_output_aliases=(),
            sim_require_finite=True,
            sim_require_nnan=True,
            nc=nc,
        )
        return tuple(outs)

    devices = jax.devices()[:NC]
    mesh = Mesh(np.asarray(devices), ("core",))
    nin = n_params + len(zero_outs)
    f = jax.jit(
        shard_map(
            _body,
            mesh=mesh,
            in_specs=(PartitionSpec("core"),) * nin,
            out_specs=(PartitionSpec("core"),) * len(out_names),
            check_rep=False,
        ),
        keep_unused=True,
    )
    concat_in = [
        np.concatenate([np.asarray(m[name]) for m in in_maps], axis=0)
        for name in in_names
    ] + [np.zeros((NC * z.shape[0], *z.shape[1:]), z.dtype) for z in zero_outs]
    sh = NamedSharding(mesh, PartitionSpec("core"))
    dev_in = [jax.device_put(a, sh) for a in concat_in]
    r = f(*dev_in)
    jax.block_until_ready(r)
    times = []
    for _ in range(n_timing):
        t0 = time.perf_counter()
        r = f(*dev_in)
        jax.block_until_ready(r)
        times.append(time.perf_counter() - t0)
    return min(times)


def _timed_run(inputs, rep_pair=(1, 3), n_timing=12, ablate=()):
    """Estimate per-iteration HW time by differencing wall times of NEFFs
    that run the whole 3-layer pipeline `r` times."""
    per_core, Tbh, tile_start, Tlo, Thi = _build_inputs_per_core(inputs)
    walls = {}
    for r in rep_pair:
        ncr = _build_bass(Tbh, tile_start, Tlo, Thi, reps=r, ablate=ablate)
        walls[r] = _exec_timed(ncr, per_core, n_timing)
    r0, r1 = rep_pair
    ns = (walls[r1] - walls[r0]) / (r1 - r0) * 1e9
    print(f"walls: {walls}", flush=True)
    return int(ns)


def _run(inputs, trace=False):
    from concourse import bass_utils

    per_core, Tbh, tile_start, Tlo, Thi = _build_inputs_per_core(inputs)
    nc = _build_bass(Tbh, tile_start, Tlo, Thi)
    res = bass_utils.run_bass_kernel_spmd(
        nc,
        per_core,
        core_ids=list(range(NC)),
        trace=trace,
    )
    outs = [r["out"][:NPC] for r in res.results]
    full = np.concatenate(outs, axis=0).astype(np.float32)
    return full, res


def kernel(**inputs):
    out, _ = _run(inputs, trace=False)
    return out


# revision 18
# speedup vs baseline: 3.9652x; 1.5442x over previous
"""Trainium2 Bass kernel for 3-layer GraphSAGE (mean aggregation).

Strategy (graph/data parallel over 8 NeuronCores):
  - Nodes are sharded contiguously: core c owns nodes [c*6250, (c+1)*6250).
  - Per layer k: every core computes m_k = h_k @ W_neigh_k.T for its own
    nodes (pre-multiplied messages, bf16, padded to 128 cols), AllGathers
    m_k into a full [50176, 128] bf16 DRAM buffer, then gathers per-edge
    source rows with dma_gather (256B rows), segment-sums them onto its
    owned destination nodes via one-hot matmuls on the PE (one-hot built
    on DVE with an iota/is_equal compare), scales by 1/deg, and adds the
    self term h_k @ W_self_k.T (+bias) computed from an on-chip transposed
    copy of h_k.
  - Graph structure (src/dst) is preprocessed on the host into per-core,
    per-destination-block edge tiles of 128, split into lo/hi halves of
    the global node-row space so gather indices fit in int16. Tile counts
    are made uniform across cores (SPMD: one NEFF for all 8 cores).
"""

import math

import numpy as np
import ml_dtypes

# problem constants (hardcoded per harness contract)
N_NODES = 50000
N_EDGES = 800000
D_IN, D_HID, D_OUT = 96, 96, 64

NC = 8  # cores
P = 128  # partitions
NPC = N_NODES // NC  # 6250 owned nodes per core
B = math.ceil(NPC / P)  # 49 dst blocks per core
NPCP = B * P  # 6272 padded nodes per core
HSPLIT = (NC // 2) * NPCP  # 25088: global row space lo/hi split
DPAD = 128  # padded message width (256B rows in bf16)
CH = 48  # gather chunk size in tiles of 128 edges

BF16 = ml_dtypes.bfloat16


def _prep_graph(src, dst):
    """Sort/pad edges into per-core, per-(block, half) tiles of 128.

    Returns per-core idx/dstid arrays plus the (core-uniform) tile counts
    Tbh[B, 2].
    """
    src = src.astype(np.int64)
    dst = dst.astype(np.int64)
    owner = dst // NPC
    dloc = dst % NPC
    blk = dloc // P
    lane = (dloc % P).astype(np.float32)
    gsrc = (src // NPC) * NPCP + (src % NPC)
    half = (gsrc >= HSPLIT).astype(np.int64)

    key = (owner * B + blk) * 2 + half
    order = np.argsort(key, kind="stable")
    gsrc_s = gsrc[order]
    lane_s = lane[order]

    counts = np.bincount(key, minlength=NC * B * 2).reshape(NC, B, 2)
    # uniform tile counts across cores; >=1 so every block has a matmul
    Tbh = np.maximum((-(-counts // P)).max(axis=0), 1)  # [B, 2]
    Tlo = int(Tbh[:, 0].sum())
    Thi = int(Tbh[:, 1].sum())
    Th = {0: Tlo, 1: Thi}
    tile_start = {
        h: np.concatenate([[0], np.cumsum(Tbh[:, h])]).astype(np.int64)
        for h in (0, 1)
    }

    starts = np.zeros(NC * B * 2 + 1, np.int64)
    np.cumsum(counts.reshape(-1), out=starts[1:])

    idx = {h: np.zeros((NC, Th[h] * P), np.int64) for h in (0, 1)}
    did = {h: np.full((NC, Th[h] * P), -1.0, np.float32) for h in (0, 1)}
    for c in range(NC):
        for b in range(B):
            for h in (0, 1):
                k = (c * B + b) * 2 + h
                s0, s1 = starts[k], starts[k + 1]
                n = s1 - s0
                off = tile_start[h][b] * P
                g = gsrc_s[s0:s1] - (HSPLIT if h else 0)
                idx[h][c, off : off + n] = g
                did[h][c, off : off + n] = lane_s[s0:s1]

    assert idx[0].max(initial=0) < 2**15 and idx[1].max(initial=0) < 2**15
    return idx, did, Tbh, tile_start, Tlo, Thi


def _wrap_idx(a):
    """[T*P] int -> dma_gather SBUF layout [128, T*8] int16 (idx i at
    [i%16, i//16], replicated to all 8 Q7-core partition groups)."""
    w = a.reshape(-1, 16).T.astype(np.int16)  # [16, T*8]
    return np.tile(w, (8, 1))


def _build_inputs_per_core(inputs):
    """Host preprocessing: shard + reorganize the problem inputs."""
    x = np.asarray(inputs["x"], np.float32)
    src = np.asarray(inputs["src"])
    dst = np.asarray(inputs["dst"])

    idx, did, Tbh, tile_start, Tlo, Thi = _prep_graph(src, dst)

    deg = np.zeros(N_NODES, np.float32)
    np.add.at(deg, dst, 1.0)
    recip = 1.0 / np.maximum(deg, 1.0)

    # weights: pre-transposed, bf16; self gets bias row appended
    wn = []
    ws = []
    for i, dout in enumerate((D_HID, D_HID, D_OUT)):
        wn.append(np.ascontiguousarray(inputs[f"w_neigh{i}"].T).astype(BF16))
        wst = np.concatenate(
            [inputs[f"w_self{i}"].T, inputs[f"b{i}"][None, :]], axis=0
        ).astype(BF16)
        ws.append(np.ascontiguousarray(wst))

    per_core = []
    for c in range(NC):
        xc = x[c * NPC : (c + 1) * NPC]
        xT = np.zeros((97, NPCP), BF16)
        xT[:96, :NPC] = xc.T.astype(BF16)
        xT[96, :] = 1.0  # ones row for the bias term
        rc = np.zeros((P, B), np.float32)
        rflat = np.zeros(NPCP, np.float32)
        rflat[:NPC] = recip[c * NPC : (c + 1) * NPC]
        rc[:, :] = rflat.reshape(B, P).T
        m = {
            "xT": xT,
            "idx_lo": _wrap_idx(idx[0][c]),
            "idx_hi": _wrap_idx(idx[1][c]),
            "dstid_lo": np.ascontiguousarray(
                did[0][c].reshape(Tlo, P).T.astype(BF16)
            ),
            "dstid_hi": np.ascontiguousarray(
                did[1][c].reshape(Thi, P).T.astype(BF16)
            ),
            "recip": rc,
        }
        for i in range(3):
            m[f"wn{i}"] = wn[i]
            m[f"ws{i}"] = ws[i]
        per_core.append(m)
    return per_core, Tbh, tile_start, Tlo, Thi


def _chunks(T):
    """Split T tiles into chunks of <=CH tiles: list of (start, count)."""
    out = []
    t = 0
    while t < T:
        ct = min(CH, T - t)
        out.append((t, ct))
        t += ct
    return out


def _build_bass(Tbh, tile_start, Tlo, Thi, reps=1, ablate=()):
    import concourse.bass as bass
    import concourse.bacc as bacc
    import concourse.mybir as mybir
    import concourse.tile as tile

    dt = mybir.dt
    Alu = mybir.AluOpType
    Act = mybir.ActivationFunctionType

    nc = bacc.Bacc(
        "TRN2",
        target_bir_lowering=False,
        debug=False,
        num_devices=NC,
        num_swdge_queues=2,
    )

    # ---- I/O ----
    xT_d = nc.dram_tensor("xT", [97, NPCP], dt.bfloat16, kind="ExternalInput")
    idx_d = {
        0: nc.dram_tensor("idx_lo", [P, Tlo * 8], dt.int16, kind="ExternalInput"),
        1: nc.dram_tensor("idx_hi", [P, Thi * 8], dt.int16, kind="ExternalInput"),
    }
    did_d = {
        0: nc.dram_tensor("dstid_lo", [P, Tlo], dt.bfloat16, kind="ExternalInput"),
        1: nc.dram_tensor("dstid_hi", [P, Thi], dt.bfloat16, kind="ExternalInput"),
    }
    recip_d = nc.dram_tensor("recip", [P, B], dt.float32, kind="ExternalInput")
    wn_d = []
    ws_d = []
    for i, dout in enumerate((D_HID, D_HID, D_OUT)):
        wn_d.append(
            nc.dram_tensor(f"wn{i}", [96, dout], dt.bfloat16, kind="ExternalInput")
        )
        ws_d.append(
            nc.dram_tensor(f"ws{i}", [97, dout], dt.bfloat16, kind="ExternalInput")
        )
    out_d = nc.dram_tensor("out", [NPCP, D_OUT], dt.float32, kind="ExternalOutput")

    ident_np = np.eye(P, dtype=BF16)
    ident_d = nc.inline_tensor(ident_np, "ident")
    iota_np = np.tile(np.arange(P, dtype=BF16)[None, :], (P, 1))
    iota_d = nc.inline_tensor(iota_np, "iota")

    # internal DRAM
    m_bounce = nc.dram_tensor("m_bounce", [NPCP, DPAD], dt.bfloat16)
    m_full = nc.dram_tensor(
        "m_full", [NC * NPCP, DPAD], dt.bfloat16, addr_space="Shared"
    )

    # ---- persistent SBUF ----
    hT = nc.alloc_sbuf_tensor("hT", [128, NPCP], dt.bfloat16)  # rows 0:97 used
    h_own = nc.alloc_sbuf_tensor("h_own", [P, B * 96], dt.bfloat16)
    m_big = nc.alloc_sbuf_tensor("m_big", [P, B * DPAD], dt.bfloat16)
    out_big = nc.alloc_sbuf_tensor("out_big", [P, B * D_OUT], dt.float32)
    idx_sb = {
        0: nc.alloc_sbuf_tensor("idx_lo_sb", [P, Tlo * 8], dt.int16),
        1: nc.alloc_sbuf_tensor("idx_hi_sb", [P, Thi * 8], dt.int16),
    }
    did_sb = {
        0: nc.alloc_sbuf_tensor("did_lo_sb", [P, Tlo], dt.bfloat16),
        1: nc.alloc_sbuf_tensor("did_hi_sb", [P, Thi], dt.bfloat16),
    }
    recip_sb = nc.alloc_sbuf_tensor("recip_sb", [P, B], dt.float32)
    ident_sb = nc.alloc_sbuf_tensor("ident_sb", [P, P], dt.bfloat16)
    iota_sb = nc.alloc_sbuf_tensor("iota_sb", [P, P], dt.bfloat16)
    wn_sb = []
    ws_sb = []
    for i, dout in enumerate((D_HID, D_HID, D_OUT)):
        wn_sb.append(nc.alloc_sbuf_tensor(f"wn{i}_sb", [96, dout], dt.bfloat16))
        ws_sb.append(nc.alloc_sbuf_tensor(f"ws{i}_sb", [97, dout], dt.bfloat16))

    douts = (D_HID, D_HID, D_OUT)
    ch_plan = {0: _chunks(Tlo), 1: _chunks(Thi)}

    with tile.TileContext(nc) as tc:
        with (
            tc.tile_pool(name="sb", bufs=2) as sb,
            tc.tile_pool(name="ps", bufs=2, space="PSUM") as ps,
        ):
            # ---- load constants / inputs to SBUF ----
            nc.vector.memset(m_big.ap(), 0)
            nc.sync.dma_start(out=hT[:97, :], in_=xT_d[:, :])
            for h in (0, 1):
                nc.sync.dma_start(out=idx_sb[h].ap(), in_=idx_d[h][:, :])
                nc.sync.dma_start(out=did_sb[h].ap(), in_=did_d[h][:, :])
            nc.sync.dma_start(out=recip_sb.ap(), in_=recip_d[:, :])
            nc.sync.dma_start(out=ident_sb.ap(), in_=ident_d[:, :])
            nc.sync.dma_start(out=iota_sb.ap(), in_=iota_d[:, :])
            for i in range(3):
                nc.sync.dma_start(out=wn_sb[i].ap(), in_=wn_d[i][:, :])
                nc.sync.dma_start(out=ws_sb[i].ap(), in_=ws_d[i][:, :])

            for rep in range(reps):
              for k in range(3):
                dout = douts[k]
                # ---- phase A: hT (layers 1,2) and m_own ----
                if k > 0 or rep > 0:
                    for b in range(B):
                        tp = ps.tile([96, P], dt.bfloat16, tag="tp")
                        nc.tensor.transpose(
                            tp[:],
                            h_own.ap()[:, b * 96 : b * 96 + 96],
                            ident_sb.ap(),
                        )
                        nc.scalar.copy(
                            out=hT[:96, b * P : (b + 1) * P], in_=tp[:]
                        )
                for b in range(B):
                    mp = ps.tile([P, dout], dt.float32, tag="pm")
                    nc.tensor.matmul(
                        mp[:],
                        hT[:96, b * P : (b + 1) * P],
                        wn_sb[k].ap(),
                        start=True,
                        stop=True,
                    )
                    nc.scalar.copy(
                        out=m_big.ap()[:, b * DPAD : b * DPAD + dout], in_=mp[:]
                    )
                # m_own -> DRAM bounce -> AllGather
                import concourse.bass as _b
                nc.sync.dma_start(
                    out=bass.AP(
                        tensor=m_bounce.ap().tensor,
                        offset=0,
                        ap=[[DPAD, P], [DPAD * P, B], [1, DPAD]],
                    ),
                    in_=m_big.ap(),
                )
                if "ag" in ablate:
                    pass
                else:
                  nc.gpsimd.collective_compute(
                    "AllGather",
                    Alu.bypass,
                    replica_groups=[list(range(NC))],
                    ins=[m_bounce.ap().opt()],
                    outs=[m_full.ap().opt()],
                )

                # ---- phase B+C: gather chunks lazily, aggregate per block ----
                mview = {
                    0: m_full[0:HSPLIT, :],
                    1: m_full[HSPLIT : 2 * HSPLIT, :],
                }
                msgs_t = {0: {}, 1: {}}
                oh_t = {0: {}, 1: {}}
                emitted = {0: 0, 1: 0}

                def emit_chunk(h):
                    ci = emitted[h]
                    t0, ct = ch_plan[h][ci]
                    ms = sb.tile([P, ct, DPAD], dt.bfloat16, tag=f"msgs{h}")
                    if "gather" in ablate:
                        nc.vector.memset(ms[:1, :1, :8], 0)
                    else:
                      nc.gpsimd.dma_gather(
                        ms[:],
                        mview[h],
                        idx_sb[h][:, t0 * 8 : (t0 + ct) * 8],
                        ct * P,
                        ct * P,
                        DPAD,
                        queue_num=h,
                        single_packet=False,
                      )
                    msgs_t[h][ci] = ms
                    oh = sb.tile([P, ct * P], dt.bfloat16, tag=f"oh{h}")
                    oh3 = oh[:].rearrange("p (t j) -> p t j", j=P)
                    nc.vector.tensor_tensor(
                        out=oh3,
                        in0=did_sb[h][:, t0 : t0 + ct].to_broadcast([P, ct, P]),
                        in1=bass.AP(
                            tensor=iota_sb.ap().tensor,
                            offset=0,
                            ap=[[P, P], [0, ct], [1, P]],
                        ),
                        op=Alu.is_equal,
                    )
                    oh_t[h][ci] = oh
                    emitted[h] = ci + 1

                for b in range(B):
                    # make sure every chunk this block touches is emitted
                    for h in (0, 1):
                        last_tile = tile_start[h][b + 1] - 1
                        while emitted[h] <= last_tile // CH:
                            emit_chunk(h)
                    pagg = ps.tile([P, dout], dt.float32, tag="pagg")
                    tiles = []
                    for h in (0, 1):
                        for o in range(tile_start[h][b], tile_start[h][b + 1]):
                            tiles.append((h, o // CH, o % CH))
                    for j, (h, ci, off) in enumerate(tiles):
                        nc.tensor.matmul(
                            pagg[:],
                            oh_t[h][ci][:, off * P : (off + 1) * P],
                            msgs_t[h][ci][:, off, :dout],
                            start=(j == 0),
                            stop=(j == len(tiles) - 1),
                        )
                    pself = ps.tile([P, dout], dt.float32, tag="pself")
                    nc.tensor.matmul(
                        pself[:],
                        hT[:97, b * P : (b + 1) * P],
                        ws_sb[k].ap(),
                        start=True,
                        stop=True,
                    )
                    selfsb = sb.tile([P, dout], dt.float32, tag="selfsb")
                    nc.scalar.copy(out=selfsb[:], in_=pself[:])
                    if k < 2:
                        t1 = sb.tile([P, dout], dt.bfloat16, tag="t1")
                        nc.vector.scalar_tensor_tensor(
                            out=t1[:],
                            in0=pagg[:],
                            scalar=recip_sb[:, b : b + 1],
                            in1=selfsb[:],
                            op0=Alu.mult,
                            op1=Alu.add,
                        )
                        nc.scalar.activation(
                            out=h_own.ap()[:, b * 96 : b * 96 + 96],
                            in_=t1[:],
                            func=Act.Relu,
                        )
                    else:
                        nc.vector.scalar_tensor_tensor(
                            out=out_big.ap()[:, b * D_OUT : (b + 1) * D_OUT],
                            in0=pagg[:],
                            scalar=recip_sb[:, b : b + 1],
                            in1=selfsb[:],
                            op0=Alu.mult,
                            op1=Alu.add,
                        )

            nc.sync.dma_start(
                out=bass.AP(
                    tensor=out_d.ap().tensor,
                    offset=0,
                    ap=[[D_OUT, P], [D_OUT * P, B], [1, D_OUT]],
                ),
                in_=out_big.ap(),
            )

    nc.finalize()
    return nc


def _exec_timed(nc, in_maps, n_timing=12):
    """Execute a prebuilt SPMD Bass module on 8 cores with device-resident
    inputs, returning the min wall-clock seconds over n_timing calls."""
    import time

    import jax
    import concourse.mybir as mybir
    from concourse import bass2jax
    from jax.experimental.shard_map import shard_map
    from jax.sharding import Mesh, NamedSharding, PartitionSpec

    bass2jax.install_neuronx_cc_hook()

    partition_name = (
        nc.partition_id_tensor.name if nc.partition_id_tensor else None
    )
    in_names, out_names, out_avals, zero_outs = [], [], [], []
    for alloc in nc.m.functions[0].allocations:
        if not isinstance(alloc, mybir.MemoryLocationSet):
            continue
        name = alloc.memorylocations[0].name
        if alloc.kind == "ExternalInput":
            if name != partition_name:
                in_names.append(name)
        elif alloc.kind == "ExternalOutput":
            dtype = mybir.dt.np(alloc.dtype)
            out_avals.append(
                jax.core.ShapedArray(tuple(alloc.tensor_shape), dtype)
            )
            out_names.append(name)
            zero_outs.append(np.zeros(tuple(alloc.tensor_shape), dtype))
    n_params = len(in_names)
    all_in_names = in_names + out_names
    if partition_name is not None:
        all_in_names = all_in_names + [partition_name]

    def _body(*args):
        operands = list(args)
        if partition_name is not None:
            operands.append(bass2jax.partition_id_tensor())
        outs = bass2jax._bass_exec_p.bind(
            *operands,
            out_avals=tuple(out_avals),
            in_names=tuple(all_in_names),
            out_names=tuple(out_names),
            lowering_input_output_aliases=(),
            sim_require_finite=True,
            sim_require_nnan=True,
            nc=nc,
        )
        return tuple(outs)

    devices = jax.devices()[:NC]
    mesh = Mesh(np.asarray(devices), ("core",))
    nin = n_params + len(zero_outs)
    f = jax.jit(
        shard_map(
            _body,
            mesh=mesh,
            in_specs=(PartitionSpec("core"),) * nin,
            out_specs=(PartitionSpec("core"),) * len(out_names),
            check_rep=False,
        ),
        keep_unused=True,
    )
    concat_in = [
        np.concatenate([np.asarray(m[name]) for m in in_maps], axis=0)
        for name in in_names
    ] + [np.zeros((NC * z.shape[0], *z.shape[1:]), z.dtype) for z in zero_outs]
    sh = NamedSharding(mesh, PartitionSpec("core"))
    dev_in = [jax.device_put(a, sh) for a in concat_in]
    r = f(*dev_in)
    jax.block_until_ready(r)
    times = []
    for _ in range(n_timing):
        t0 = time.perf_counter()
        r = f(*dev_in)
        jax.block_until_ready(r)
        times.append(time.perf_counter() - t0)
    return min(times)


def _timed_run(inputs, rep_pair=(1, 3), n_timing=12, ablate=()):
    """Estimate per-iteration HW time by differencing wall times of NEFFs
    that run the whole 3-layer pipeline `r` times."""
    per_core, Tbh, tile_start, Tlo, Thi = _build_inputs_per_core(inputs)
    ncs = {
        r: _build_bass(Tbh, tile_start, Tlo, Thi, reps=r, ablate=ablate)
        for r in rep_pair
    }
    # the axon RPC wall floor is bimodal across batches; alternate batches
    # and take the global min per rep so both reps see the fast mode
    walls = {r: [] for r in rep_pair}
    for _ in range(3):
        for r in rep_pair:
            walls[r].append(_exec_timed(ncs[r], per_core, n_timing))
    r0, r1 = rep_pair
    ns = (min(walls[r1]) - min(walls[r0])) / (r1 - r0) * 1e9
    print(f"walls: { {r: [f'{w * 1e3:.2f}ms' for w in v] for r, v in walls.items()} }", flush=True)
    return int(ns)


def _run(inputs, trace=False):
    from concourse import bass_utils

    per_core, Tbh, tile_start, Tlo, Thi = _build_inputs_per_core(inputs)
    nc = _build_bass(Tbh, tile_start, Tlo, Thi)
    res = bass_utils.run_bass_kernel_spmd(
        nc,
        per_core,
        core_ids=list(range(NC)),
        trace=trace,
    )
    outs = [r["out"][:NPC] for r in res.results]
    full = np.concatenate(outs, axis=0).astype(np.float32)
    return full, res


def kernel(**inputs):
    out, _ = _run(inputs, trace=False)
    return out
